# revision 23
# baseline (speedup 1.0000x reference)
"""BPaCo+ loss on 8 TRN2 NeuronCores.

Flipped-layout design: anchors (i) on PSUM partitions, queue columns (j) on
the free axis. Each core owns K/8 = 4096 queue columns. One fp8e4m3 DoubleRow
matmul per PSUM bank computes, in a single pass,
    P[i,j] = f_i.f_j/T + ln(r0_j) + shift
by packing the 128 feature rows plus a rank-1 log-weight row (129 rows) into
65 partitions x 2 (DoubleRow virtualizes the contraction to 130). The scalar
engine exps [128, 2048] PSUM tiles into bf16 SBUF; the vector and gpsimd
engines each free-axis-reduce half of every exp tile into the softmax
denominator partials (the final tile instead uses the activation accumulator
so the tail is one DMA away). Queue columns are never diagonal and their
class-matched weight correction ln(c/(c-alpha)) ~ 1.5e-4 is dropped.

The O(B*(B+C)) blocks (batch-vs-batch, branch 1, sup logits) and the final
log/mean run on host in exact arithmetic. Device returns [128, 32] partial
sums per core; host gathers, adds its blocks, and assembles the loss.
"""
import numpy as np
import ml_dtypes

from concourse import bass, bacc, mybir, tile
from concourse.bass_utils import run_bass_kernel_spmd

B, K, C, D = 1024, 32768, 100, 128
T, ALPHA = 0.07, 0.05
M = 8                       # cores
QSH = K // M                # 4096 queue cols per core
IB = 8                      # anchor i-blocks of 128
TLE = 2048                  # exp tile columns
NT = QSH // TLE             # 2 col-tiles per iblock
CH = 1024                   # DMA chunk columns

BF16 = mybir.dt.bfloat16
F32 = mybir.dt.float32
FP8 = mybir.dt.float8e4
NP_BF16 = ml_dtypes.bfloat16
NP_FP8 = ml_dtypes.float8_e4m3

_CACHE = {}


def _build_nc():
    nc = bacc.Bacc(None, target_bir_lowering=False)
    X0 = nc.declare_dram_parameter("X0", [65, 2, 128], FP8, isOutput=False)
    XR = nc.declare_dram_parameter("XR", [65, 2, (IB - 1) * 128], FP8, isOutput=False)
    RQ = [
        nc.declare_dram_parameter(f"RQ{k}", [65, 2, CH], FP8, isOutput=False)
        for k in range(QSH // CH)
    ]
    ACC = nc.declare_dram_parameter("ACC", [128, 16], F32, isOutput=True)

    with tile.TileContext(nc) as tc:
        with (
            tc.tile_pool(name="sb", bufs=1) as sbp,
            tc.tile_pool(name="ps", bufs=2, space=bass.MemorySpace.PSUM) as pps,
        ):
            # parallel DMA issue, first-needed data on the fast scalar queue,
            # bulk on the eager gpsimd SWDGE path, sync kept light
            RQ_sb = [
                sbp.tile([65, 2, CH], FP8, tag=f"RQ{k}", name=f"RQ{k}_sb")
                for k in range(QSH // CH)
            ]
            X0_sb = sbp.tile([65, 2, 128], FP8, tag="X0")
            XR_sb = sbp.tile([65, 2, (IB - 1) * 128], FP8, tag="XR")
            nc.scalar.dma_start(X0_sb[:], X0[:])
            nc.scalar.dma_start(RQ_sb[0][:], RQ[0][:])
            nc.sync.dma_start(RQ_sb[1][:], RQ[1][:])
            nc.gpsimd.dma_start(XR_sb[:], XR[:])
            nc.gpsimd.dma_start(RQ_sb[2][:], RQ[2][:])
            nc.gpsimd.dma_start(RQ_sb[3][:], RQ[3][:])

            ACC_sb = sbp.tile([128, 16], F32, tag="ACCsb")
            warm = sbp.tile([128, 1], F32, tag="warm")
            nc.gpsimd.memset(warm[:], 0.0)
            # pre-load the Exp activation table while DMAs are in flight
            nc.scalar.activation(
                warm[:], warm[:], mybir.ActivationFunctionType.Exp)

            DR = mybir.MatmulPerfMode.DoubleRow
            ACCUM_TILES = {2, 5, 8, 11, 15}
            for t in range(NT * IB):
                ct, b = t // IB, t % IB
                lhs = X0_sb[:] if b == 0 else XR_sb[:, :, (b - 1) * 128:b * 128]
                P = pps.tile([128, TLE], F32, tag="P")
                for h in range(TLE // CH):
                    RQh = RQ_sb[ct * (TLE // CH) + h]
                    for c0 in range(0, CH, 512):
                        nc.tensor.matmul(
                            P[:, h * CH + c0:h * CH + c0 + 512],
                            lhs,
                            RQh[:, :, c0:c0 + 512],
                            start=True, stop=True,
                            perf_mode=DR,
                        )
                E = sbp.tile([128, TLE], BF16, tag="E", bufs=3)
                use_accum = t in ACCUM_TILES
                nc.scalar.activation(
                    E[:], P[:], mybir.ActivationFunctionType.Exp,
                    accum_out=ACC_sb[:, t:t + 1] if use_accum else None,
                )
                if not use_accum:
                    nc.vector.tensor_reduce(
                        ACC_sb[:, t:t + 1], E[:],
                        axis=mybir.AxisListType.X, op=mybir.AluOpType.add,
                    )

            nc.sync.dma_start(ACC[:], ACC_sb[:])

    nc.compile()
    return nc


def _prep_inputs(features, labels):
    f = features.astype(np.float64)
    lab = labels.astype(np.int64)
    ccount = np.bincount(lab, minlength=C).astype(np.float64)

    lnr0 = -np.log(ccount)
    s2 = -np.median(lnr0[lab])
    lnr0p = lnr0[lab] + s2

    fq = f.astype(NP_FP8).astype(np.float32)
    fTq = (f[:B] / T).astype(NP_FP8).astype(np.float32)

    lx = np.zeros((130, B), np.float32)
    lx[:D] = fTq.T
    lx[D] = 1.0
    X = np.ascontiguousarray(lx.reshape(65, 2, B)).astype(NP_FP8)
    X0 = np.ascontiguousarray(X[:, :, :128])
    XR = np.ascontiguousarray(X[:, :, 128:])

    in_maps = []
    for c in range(M):
        rq = np.zeros((130, QSH), np.float32)
        jQ = slice(B + c * QSH, B + (c + 1) * QSH)
        rq[:D] = fq[jQ].T
        rq[D] = lnr0p[jQ]
        rq = rq.reshape(65, 2, QSH).astype(NP_FP8)
        im = {"X0": X0, "XR": XR}
        for k in range(QSH // CH):
            im[f"RQ{k}"] = np.ascontiguousarray(rq[:, :, k * CH:(k + 1) * CH])
        in_maps.append(im)
    return in_maps, s2


def kernel(features, sup_logits, centers, labels, _debug=False, _trace=False):
    if "nc" not in _CACHE:
        _CACHE["nc"] = _build_nc()
    nc = _CACHE["nc"]
    in_maps, s2 = _prep_inputs(features, labels)
    res = run_bass_kernel_spmd(nc, in_maps, core_ids=list(range(M)), trace=_trace)
    _CACHE["last"] = res

    acc = np.zeros((128, 16), np.float64)
    for c in range(M):
        acc += res.results[c]["ACC"].astype(np.float64)
    # tile t = ct*IB + b covers anchors i = 128b+p
    per_block = acc[:, 0:IB] + acc[:, IB:2 * IB]  # [128 lane, 8 iblock]
    S2q = per_block.T.reshape(B) * np.exp(-s2)

    # ---- host blocks (exact): batch-vs-batch, branch 1, sup logits ----
    f = features.astype(np.float64)
    f32b = features.astype(np.float32)
    sup = sup_logits.astype(np.float64)
    lab = labels.astype(np.int64)
    labB = lab[:B]
    ccount = np.bincount(lab, minlength=C).astype(np.float64)
    cntB = np.bincount(labB, minlength=C).astype(np.float64)
    cc1 = cntB + 1.0

    cols = np.concatenate([f32b[:B], centers.astype(np.float32)], axis=0)  # [B+C, D]
    LG = (f32b[:B] @ cols.T) / np.float32(T)          # [B, B+C]
    ELG = np.exp(LG.astype(np.float64))
    ELG[np.arange(B), np.arange(B)] = 0.0             # diag masked in both branches

    match_bb = labB[:, None] == labB[None, :]
    W2 = 1.0 / (ccount[labB][None, :] - ALPHA * match_bb)
    S2h = (ELG[:, :B] * W2).sum(1)
    oh = labB[:, None] == np.arange(C)[None, :]
    S2sup = (np.exp(sup) / (ccount[None, :] - oh)).sum(1)
    S2 = S2q + S2h + S2sup

    lab1 = np.concatenate([labB, np.arange(C)])
    match1 = labB[:, None] == lab1[None, :]
    W1 = 1.0 / (cc1[lab1][None, :] - match1)  # diag already zeroed in ELG
    S1 = (ELG * W1).sum(1)

    g2 = np.zeros((C, D))
    np.add.at(g2, lab, f)
    g1 = np.zeros((C, D))
    np.add.at(g1, labB, f[:B])
    g1 += centers.astype(np.float64)
    A2 = np.einsum("id,id->i", f[:B], g2[labB]) / T - 1.0 / T
    A1 = np.einsum("id,id->i", f[:B], g1[labB]) / T - 1.0 / T

    msum = 1.0 + ALPHA * (ccount[labB] - 1.0)
    numer2 = sup[np.arange(B), labB] + ALPHA * A2
    loss2 = np.mean(np.log(S2) - numer2 / msum)
    loss1 = np.mean(np.log(S1) - A1 / cntB[labB])
    return np.array(loss1 + loss2, dtype=np.float32)
